# revision 1
# baseline (speedup 1.0000x reference)
"""Trainium2 Bass kernel for InvariantMessage GNN message passing.

out[e, :] = (MLP(s_j)[nbrs[e,1]]) * ((rbf(dist[e]) @ W_rbf + b_rbf) * env(dist[e]))

Strategy (8 cores, edge-parallel):
- Each core redundantly computes inv = MLP(s_j) for all nodes into two DRAM
  tables (invA: nodes < 32768, invB: rest).
- Edges are sharded 100k/core; the host pre-partitions each shard into
  low/high node-index groups (gathered from invA/invB), pads each group to a
  uniform number of 3072-edge chunks across cores (pad slots gather row 0 and
  are discarded), and precomputes broadcast/transposed dist layouts. Gathers
  use indirect_dma_start, 128 rows (one per partition) per instruction --
  the HW-validated semantics (dma_gather int16 hung the device under Tile).
- Per 512 edges: sin args n*pi*d/5 are range-reduced to [-pi, pi] with the
  fp32 magic-number rounding trick, evaluated by ScalarE Sin in a 4x(32-row)
  quadrant-packed layout [sin rows 0..19 + raw d row 20 per group], then a
  K=21 matmul against [W_rbf; b_rbf] gives w*d in PSUM; scaling by env/d and
  the gathered phi finishes the edge.
"""
import sys

sys.path.insert(0, "/opt/trn_rl_repo")

import numpy as np

import concourse.tile as tile
from concourse import bass, bacc, mybir
from concourse.bass_utils import run_bass_kernel_spmd

F32 = mybir.dt.float32
I16 = mybir.dt.int16

N_CORES = 8
N_ATOMS = 50000
N_EDGES = 800000
D = 128
NB = 20
CUTOFF = 5.0
MAGIC = float(np.float32(1.5 * 2**23))

SPLIT = 32768                     # invA rows; invB = rest
E_CORE = N_EDGES // N_CORES       # 100000
GCH = 3072                        # edges per dma_gather / out-dma chunk
NODE_PAD = 50176                  # 98 * 512
NCH_NODE = NODE_PAD // 512        # 98
VB = NODE_PAD - SPLIT             # 17408 invB rows (34*512)
SPLIT_CHUNK = SPLIT // 512        # 64


def build_nc(n_gchunks, low_chunks):
    """Build the Bass program: chunk g gathers from invA if g < low_chunks
    else invB; every chunk is full (GCH valid indices)."""
    nc = bacc.Bacc(None, target_bir_lowering=False)
    E_pad = n_gchunks * GCH

    s_jT = nc.dram_tensor("s_jT", [D, NODE_PAD], F32, kind="ExternalInput")
    W1 = nc.dram_tensor("W1", [D, D], F32, kind="ExternalInput")
    W2 = nc.dram_tensor("W2", [D, D], F32, kind="ExternalInput")
    b1c = nc.dram_tensor("b1c", [D, 1], F32, kind="ExternalInput")
    b2c = nc.dram_tensor("b2c", [D, 1], F32, kind="ExternalInput")
    wext = nc.dram_tensor("wext", [D, D], F32, kind="ExternalInput")
    coef2 = nc.dram_tensor("coef2", [D, 1], F32, kind="ExternalInput")
    ident = nc.dram_tensor("ident", [D, D], F32, kind="ExternalInput")
    nhp = nc.dram_tensor("nhp", [D, 1], F32, kind="ExternalInput")
    dist_b = nc.dram_tensor("dist_b", [(E_pad // 384) * D, D], F32,
                            kind="ExternalInput")
    dist_t2 = nc.dram_tensor("dist_t2", [(E_pad // 1536) * D, 12], F32,
                             kind="ExternalInput")
    idx32 = nc.dram_tensor("idx32", [n_gchunks * GCH, 1], mybir.dt.int32,
                           kind="ExternalInput")
    out_dev = nc.dram_tensor("out_dev", [E_pad, D], F32, kind="ExternalOutput")

    invA = nc.dram_tensor("invA", [SPLIT, D], F32, kind="ExternalInput")
    invB = nc.dram_tensor("invB", [VB, D], F32, kind="ExternalInput")

    with tile.TileContext(nc) as tc:
        with tc.tile_pool(name="const", bufs=1) as cpool, \
             tc.tile_pool(name="mlp", bufs=3) as mpool, \
             tc.tile_pool(name="mlppsum", bufs=1, space="PSUM") as mpsum, \
             tc.tile_pool(name="tpsum", bufs=2, space="PSUM") as tpsum, \
             tc.tile_pool(name="edge", bufs=3) as epool, \
             tc.tile_pool(name="big", bufs=2) as bpool, \
             tc.tile_pool(name="wpsum", bufs=4, space="PSUM") as wpsum:

            w1_sb = cpool.tile([D, D], F32)
            nc.sync.dma_start(out=w1_sb[:], in_=W1[:])
            w2_sb = cpool.tile([D, D], F32)
            nc.sync.dma_start(out=w2_sb[:], in_=W2[:])
            b1_sb = cpool.tile([D, 1], F32)
            nc.sync.dma_start(out=b1_sb[:], in_=b1c[:])
            b2_sb = cpool.tile([D, 1], F32)
            nc.sync.dma_start(out=b2_sb[:], in_=b2c[:])
            wext_sb = cpool.tile([D, D], F32)
            nc.sync.dma_start(out=wext_sb[:], in_=wext[:])
            coef_sb = cpool.tile([D, 1], F32)
            nc.sync.dma_start(out=coef_sb[:], in_=coef2[:])
            id_sb = cpool.tile([D, D], F32)
            nc.sync.dma_start(out=id_sb[:], in_=ident[:])
            nhp_sb = cpool.tile([D, 1], F32)
            nc.sync.dma_start(out=nhp_sb[:], in_=nhp[:])

            # ---- Phase 1: node MLP -> invA / invB ----
            for i in range(NCH_NODE):
                s_t = mpool.tile([D, 512], F32, tag="s")
                nc.sync.dma_start(out=s_t[:], in_=s_jT[:, i * 512:(i + 1) * 512])
                ph = mpsum.tile([D, 512], F32, tag="ph")
                nc.tensor.matmul(out=ph[:], lhsT=w1_sb[:], rhs=s_t[:],
                                 start=True, stop=True)
                h_t = mpool.tile([D, 512], F32, tag="h")
                nc.scalar.activation(out=h_t[:], in_=ph[:],
                                     func=mybir.ActivationFunctionType.Silu,
                                     bias=b1_sb[:, 0:1], scale=1.0)
                pi = mpsum.tile([D, 512], F32, tag="pi")
                nc.tensor.matmul(out=pi[:], lhsT=w2_sb[:], rhs=h_t[:],
                                 start=True, stop=True)
                iv = mpool.tile([D, 512], F32, tag="iv")
                nc.vector.tensor_scalar_add(out=iv[:], in0=pi[:],
                                            scalar1=b2_sb[:, 0:1])
                for j in range(4):
                    pt = tpsum.tile([D, D], F32, tag="pt")
                    nc.tensor.transpose(out=pt[:], in_=iv[:, j * D:(j + 1) * D],
                                        identity=id_sb[:])
                    ot = mpool.tile([D, D], F32, tag="ot")
                    nc.scalar.copy(out=ot[:], in_=pt[:])
                    n0 = i * 512 + j * D
                    if i < SPLIT_CHUNK:
                        nc.sync.dma_start(out=invA[n0:n0 + D, :], in_=ot[:])
                    else:
                        m0 = n0 - SPLIT
                        nc.sync.dma_start(out=invB[m0:m0 + D, :], in_=ot[:])

            # ---- Phase 2: edges ----
            for g in range(n_gchunks):
                ix = epool.tile([D, GCH // D], mybir.dt.int32, tag="ix")
                nc.sync.dma_start(
                    out=ix[:],
                    in_=idx32[g * GCH:(g + 1) * GCH, :].rearrange(
                        "(s p) o -> p (s o)", p=D))
                phi = bpool.tile([D, GCH // D, D], F32, tag="phi")
                table = invA if g < low_chunks else invB
                for s in range(GCH // D):
                    nc.gpsimd.indirect_dma_start(
                        out=phi[:, s, :], out_offset=None, in_=table[:],
                        in_offset=bass.IndirectOffsetOnAxis(
                            ap=ix[:, s:s + 1], axis=0))
                out_sb = bpool.tile([D, GCH // D, D], F32, tag="osb")
                for c2 in range(GCH // 1536):
                    e0 = g * GCH + c2 * 1536
                    cb0 = e0 // 384
                    db = epool.tile([D, 512], F32, tag="db")
                    for k in range(4):
                        nc.sync.dma_start(
                            out=db[:, k * D:(k + 1) * D],
                            in_=dist_b[(cb0 + k) * D:(cb0 + k + 1) * D, :])
                    dt = epool.tile([D, 12], F32, tag="dt")
                    c2g = e0 // 1536
                    nc.sync.dma_start(out=dt[:],
                                      in_=dist_t2[c2g * D:(c2g + 1) * D, :])
                    u = epool.tile([D, 512], F32, tag="u")
                    nc.scalar.activation(out=u[:], in_=db[:],
                                         func=mybir.ActivationFunctionType.Copy,
                                         scale=coef_sb[:, 0:1])
                    kf = epool.tile([D, 512], F32, tag="kf")
                    nc.vector.tensor_scalar(out=kf[:], in0=u[:],
                                            scalar1=MAGIC, scalar2=MAGIC,
                                            op0=mybir.AluOpType.add,
                                            op1=mybir.AluOpType.subtract)
                    v = epool.tile([D, 512], F32, tag="v")
                    nc.vector.tensor_tensor(out=v[:], in0=u[:], in1=kf[:],
                                            op=mybir.AluOpType.subtract)
                    for j in range(3):
                        nc.scalar.activation(
                            out=db[32 * j:32 * j + NB, :],
                            in_=v[32 * j:32 * j + NB, :],
                            func=mybir.ActivationFunctionType.Sin,
                            scale=float(2 * np.pi))
                    rd = epool.tile([D, 12], F32, tag="rd")
                    nc.vector.reciprocal(out=rd[:], in_=dt[:])
                    cs = epool.tile([D, 12], F32, tag="cs")
                    nc.scalar.activation(out=cs[:], in_=dt[:],
                                         func=mybir.ActivationFunctionType.Sin,
                                         scale=float(np.pi / CUTOFF),
                                         bias=nhp_sb[:, 0:1])
                    env = epool.tile([D, 12], F32, tag="env")
                    nc.vector.tensor_scalar(out=env[:], in0=cs[:],
                                            scalar1=-0.5, scalar2=0.5,
                                            op0=mybir.AluOpType.mult,
                                            op1=mybir.AluOpType.add)
                    scl = epool.tile([D, 12], F32, tag="scl")
                    nc.vector.tensor_tensor(out=scl[:], in0=env[:], in1=rd[:],
                                            op=mybir.AluOpType.mult)
                    for t in range(12):
                        k, j = t // 3, t % 3
                        pw = wpsum.tile([D, D], F32, tag="pw")
                        nc.tensor.matmul(
                            out=pw[:],
                            lhsT=db[32 * j:32 * j + NB + 1, k * D:(k + 1) * D],
                            rhs=wext_sb[32 * j:32 * j + NB + 1, :],
                            start=True, stop=True)
                        ws = epool.tile([D, D], F32, tag="ws")
                        nc.scalar.activation(
                            out=ws[:], in_=pw[:],
                            func=mybir.ActivationFunctionType.Copy,
                            scale=scl[:, t:t + 1])
                        slot = c2 * 12 + t
                        nc.vector.tensor_tensor(
                            out=out_sb[:, slot, :], in0=ws[:],
                            in1=phi[:, slot, :], op=mybir.AluOpType.mult)
                nc.sync.dma_start(
                    out=out_dev[g * GCH:(g + 1) * GCH, :].rearrange(
                        "(s p) f -> p s f", p=D),
                    in_=out_sb[:])
    nc.finalize()
    return nc


_NC_CACHE = {}


def kernel(s_j, dist, nbrs, W1, b1, W2, b2, W_rbf, b_rbf):
    s_j = np.asarray(s_j, dtype=np.float32)
    dist = np.asarray(dist, dtype=np.float32)
    idx_all = np.asarray(nbrs)[:, 1].astype(np.int32)

    s_jT = np.zeros((D, NODE_PAD), dtype=np.float32)
    s_jT[:, :N_ATOMS] = s_j.T
    w21 = np.concatenate([np.asarray(W_rbf, np.float32),
                          np.asarray(b_rbf, np.float32)[None, :]], axis=0)
    wext = np.zeros((D, D), dtype=np.float32)
    for qj in range(3):
        wext[32 * qj:32 * qj + NB + 1] = w21
    coef2 = np.zeros((D, 1), dtype=np.float32)
    for p in range(96):
        n = p % 32
        if n < NB:
            coef2[p, 0] = (n + 1) / 10.0
    common = {
        "s_jT": s_jT,
        "W1": np.asarray(W1, np.float32),
        "W2": np.asarray(W2, np.float32),
        "b1c": np.asarray(b1, np.float32).reshape(D, 1),
        "b2c": np.asarray(b2, np.float32).reshape(D, 1),
        "wext": wext,
        "coef2": coef2,
        "ident": np.eye(D, dtype=np.float32),
        "nhp": np.full((D, 1), -np.pi / 2, dtype=np.float32),
        "invA": np.zeros((SPLIT, D), dtype=np.float32),
        "invB": np.zeros((VB, D), dtype=np.float32),
    }

    # shard stats -> uniform chunk counts across cores
    shards = []
    for c in range(N_CORES):
        sl = slice(c * E_CORE, (c + 1) * E_CORE)
        ish, dsh = idx_all[sl], dist[sl]
        low = ish < SPLIT
        shards.append((ish, dsh, low, int(low.sum())))
    max_low = max(s[3] for s in shards)
    max_high = max(E_CORE - s[3] for s in shards)
    LC = (max_low + GCH - 1) // GCH        # low chunks
    HC = (max_high + GCH - 1) // GCH       # high chunks
    n_g = LC + HC
    LP = LC * GCH
    E_pad = n_g * GCH

    in_maps, metas = [], []
    for c in range(N_CORES):
        ish, dsh, low, n_low = shards[c]
        n_high = E_CORE - n_low
        perm = np.argsort(~low, kind="stable")
        idx_pad = np.zeros(E_pad, dtype=np.int32)
        dist_pad = np.ones(E_pad, dtype=np.float32)
        idx_p, dist_p = ish[perm], dsh[perm]
        idx_pad[:n_low] = idx_p[:n_low]
        dist_pad[:n_low] = dist_p[:n_low]
        idx_pad[LP:LP + n_high] = idx_p[n_low:] - SPLIT
        dist_pad[LP:LP + n_high] = dist_p[n_low:]
        # dist_b[c*128+p, e] = dist_pad[c*384 + (p//32)*128 + e], p<96
        dseg = dist_pad.reshape(-1, 3, 128)
        db3 = np.repeat(dseg, 32, axis=1)
        db = np.concatenate(
            [db3, np.ones((db3.shape[0], 32, 128), np.float32)],
            axis=1).reshape(-1, 128)
        # dist_t2[c2*128+p, t] = dist_pad[c2*1536 + t*128 + p]
        dt2 = np.ascontiguousarray(
            dist_pad.reshape(-1, 12, 128).transpose(0, 2, 1).reshape(-1, 12))
        metas.append((perm, n_low, n_high))
        in_maps.append(dict(common, dist_b=db, dist_t2=dt2,
                            idx32=idx_pad.reshape(-1, 1)))

    ckey = (n_g, LC)
    if ckey not in _NC_CACHE:
        _NC_CACHE[ckey] = build_nc(n_g, LC)
    nc = _NC_CACHE[ckey]

    res = run_bass_kernel_spmd(nc, in_maps, list(range(N_CORES)))
    out = np.empty((N_EDGES, D), dtype=np.float32)
    for c in range(N_CORES):
        perm, n_low, n_high = metas[c]
        od = res.results[c]["out_dev"]
        shard = np.empty((E_CORE, D), dtype=np.float32)
        shard[perm] = np.concatenate([od[:n_low], od[LP:LP + n_high]], axis=0)
        out[c * E_CORE:(c + 1) * E_CORE] = shard
    return out



# revision 22
# speedup vs baseline: 7.2055x; 7.2055x over previous
"""Trainium2 Bass kernel for InvariantMessage GNN message passing.

out[e, :] = (MLP(s_j)[nbrs[e,1]]) * ((rbf(dist[e]) @ W_rbf + b_rbf) * env(dist[e]))

The axon tunnel (~30-90 MB/s) dominates wall time, so the design minimizes
host<->device bytes; on-device compute is ~0.1 s.

Strategy (8 cores, node-sharded):
- Nodes are split 6250/core; each EDGE is assigned to the core that owns its
  gathered node nbrs[e,1], so every gather is core-local (no collectives) and
  s_j is uploaded exactly once across the fleet (fp16, 1.7 MB/core).
- Each core runs the MLP on its 6250 nodes into an Internal-DRAM table
  (no zero upload), then per 3072-edge chunk gathers phi rows with
  indirect_dma_start (128 rows/instruction -- the HW-validated semantics).
- dist is uploaded once in a [12,128]-per-1536-edge layout; the quadrant-
  packed sin-argument tile is built on device by K=3 outer-product matmuls
  against a coefficient matrix (row n<20: (n+1)/10, row 20: 1.0 for the raw-d
  bias row), range-reduced with the fp32 magic-number trick, evaluated by
  ScalarE Sin, then a K=21 matmul against [W_rbf; b_rbf] gives w*d in PSUM;
  scaling by env/d and the gathered phi finishes the edge.
- Output is written fp16 (halves both the donated-zero upload and the
  fetch) and upcast on host; worst-case per-element error ~0.3%.
"""
import sys

sys.path.insert(0, "/opt/trn_rl_repo")

import numpy as np

import concourse.tile as tile
from concourse import bass, bacc, mybir
from concourse.bass_utils import run_bass_kernel_spmd

F32 = mybir.dt.float32
F16 = mybir.dt.float16
I32 = mybir.dt.int32

N_CORES = 8
N_ATOMS = 50000
N_EDGES = 800000
D = 128
NB = 20
CUTOFF = 5.0
MAGIC = float(np.float32(1.5 * 2**23))

NPC = N_ATOMS // N_CORES          # 6250 nodes per core
NPT = 6656                        # table rows = 13*512 (>= NPC)
NCH_NODE = NPT // 512             # 13 node MLP chunks
GCH = 3072                        # edges per gather/output chunk


def build_nc(n_gchunks):
    nc = bacc.Bacc(None, target_bir_lowering=False)
    E_pad = n_gchunks * GCH

    s_jT = nc.dram_tensor("s_jT", [D, NPT], F16, kind="ExternalInput")
    W1h = nc.dram_tensor("W1h", [D, D], F16, kind="ExternalInput")
    W2h = nc.dram_tensor("W2h", [D, D], F16, kind="ExternalInput")
    b1c = nc.dram_tensor("b1c", [D, 1], F32, kind="ExternalInput")
    ones1 = nc.dram_tensor("ones1", [1, D], F16, kind="ExternalInput")
    b2r = nc.dram_tensor("b2r", [1, D], F16, kind="ExternalInput")
    wext = nc.dram_tensor("wext", [D, D], F32, kind="ExternalInput")
    cmat = nc.dram_tensor("cmat", [12, 512], F32, kind="ExternalInput")
    ident = nc.dram_tensor("ident", [D, D], F32, kind="ExternalInput")
    nhp = nc.dram_tensor("nhp", [D, 1], F32, kind="ExternalInput")
    idx32 = nc.dram_tensor("idx32", [E_pad, 1], I32, kind="ExternalInput")
    dist3 = nc.dram_tensor("dist3", [E_pad // D, D], F32, kind="ExternalInput")
    out_dev = nc.dram_tensor("out_dev", [E_pad, D], F16, kind="ExternalOutput")

    inv = nc.dram_tensor("inv", [NPT, D], F32, kind="Internal")

    with tile.TileContext(nc) as tc:
        with tc.tile_pool(name="const", bufs=1) as cpool, \
             tc.tile_pool(name="mlp", bufs=3) as mpool, \
             tc.tile_pool(name="bigp", bufs=2, space="PSUM") as bigp, \
             tc.tile_pool(name="smallp", bufs=4, space="PSUM") as smallp, \
             tc.tile_pool(name="dtpp", bufs=2, space="PSUM") as dtpp, \
             tc.tile_pool(name="edge", bufs=3) as epool, \
             tc.tile_pool(name="big", bufs=2) as bpool:

            w1_sb = cpool.tile([D, D], F16)
            nc.sync.dma_start(out=w1_sb[:], in_=W1h[:])
            w2_sb = cpool.tile([D, D], F16)
            nc.sync.dma_start(out=w2_sb[:], in_=W2h[:])
            b1_sb = cpool.tile([D, 1], F32)
            nc.sync.dma_start(out=b1_sb[:], in_=b1c[:])
            ones_sb = cpool.tile([1, D], F16)
            nc.sync.dma_start(out=ones_sb[:], in_=ones1[:])
            b2r_sb = cpool.tile([1, D], F16)
            nc.sync.dma_start(out=b2r_sb[:], in_=b2r[:])
            wext_sb = cpool.tile([D, D], F32)
            nc.sync.dma_start(out=wext_sb[:], in_=wext[:])
            cm_sb = cpool.tile([12, 512], F32)
            nc.sync.dma_start(out=cm_sb[:], in_=cmat[:])
            id_sb = cpool.tile([D, D], F32)
            nc.sync.dma_start(out=id_sb[:], in_=ident[:])
            nhp_sb = cpool.tile([D, 1], F32)
            nc.sync.dma_start(out=nhp_sb[:], in_=nhp[:])

            # ---- Phase 1: node MLP -> inv table (node-major rows) ----
            for i in range(NCH_NODE):
                s_t = mpool.tile([D, 512], F16, tag="s")
                nc.sync.dma_start(out=s_t[:], in_=s_jT[:, i * 512:(i + 1) * 512])
                ph = bigp.tile([D, 512], F32, tag="mm512")
                nc.tensor.matmul(out=ph[:], lhsT=w1_sb[:], rhs=s_t[:],
                                 start=True, stop=True)
                h_t = mpool.tile([D, 512], F16, tag="h")
                nc.scalar.activation(out=h_t[:], in_=ph[:],
                                     func=mybir.ActivationFunctionType.Silu,
                                     bias=b1_sb[:, 0:1], scale=1.0)
                for b in range(4):
                    pt = smallp.tile([D, D], F32, tag="mm128")
                    nc.tensor.matmul(out=pt[:],
                                     lhsT=h_t[:, b * D:(b + 1) * D],
                                     rhs=w2_sb[:], start=True, stop=False)
                    nc.tensor.matmul(out=pt[:], lhsT=ones_sb[:],
                                     rhs=b2r_sb[:], start=False, stop=True)
                    ot = mpool.tile([D, D], F32, tag="ot")
                    nc.scalar.copy(out=ot[:], in_=pt[:])
                    n0 = i * 512 + b * D
                    nc.sync.dma_start(out=inv[n0:n0 + D, :], in_=ot[:])

            # ---- Phase 2: edges ----
            for g in range(n_gchunks):
                ix = epool.tile([D, GCH // D], I32, tag="ix")
                nc.sync.dma_start(
                    out=ix[:],
                    in_=idx32[g * GCH:(g + 1) * GCH, :].rearrange(
                        "(s p) o -> p (s o)", p=D))
                phi = bpool.tile([D, GCH // D, D], F32, tag="phi")
                for s in range(GCH // D):
                    nc.gpsimd.indirect_dma_start(
                        out=phi[:, s, :], out_offset=None, in_=inv[:],
                        in_offset=bass.IndirectOffsetOnAxis(
                            ap=ix[:, s:s + 1], axis=0))
                out_sb = bpool.tile([D, GCH // D, D], F16, tag="osb")
                for c2 in range(GCH // 1536):
                    blk = g * 2 + c2
                    # dist rows for this 1536-edge block: [12, 128]
                    dt3 = epool.tile([12, D], F32, tag="dt3")
                    nc.sync.dma_start(out=dt3[:],
                                      in_=dist3[blk * 12:(blk + 1) * 12, :])
                    # transpose -> [128, 12] per-partition dist scalars
                    dtp = dtpp.tile([D, 12], F32, tag="dtp")
                    nc.tensor.transpose(out=dtp[:], in_=dt3[:],
                                        identity=id_sb[0:12, 0:12])
                    dt = epool.tile([D, 12], F32, tag="dt")
                    nc.scalar.copy(out=dt[:], in_=dtp[:])
                    rd = epool.tile([D, 12], F32, tag="rd")
                    nc.vector.reciprocal(out=rd[:], in_=dt[:])
                    cs = epool.tile([D, 12], F32, tag="cs")
                    nc.scalar.activation(out=cs[:], in_=dt[:],
                                         func=mybir.ActivationFunctionType.Sin,
                                         scale=float(np.pi / CUTOFF),
                                         bias=nhp_sb[:, 0:1])
                    env = epool.tile([D, 12], F32, tag="env")
                    nc.vector.tensor_scalar(out=env[:], in0=cs[:],
                                            scalar1=-0.5, scalar2=0.5,
                                            op0=mybir.AluOpType.mult,
                                            op1=mybir.AluOpType.add)
                    scl = epool.tile([D, 12], F32, tag="scl")
                    nc.vector.tensor_tensor(out=scl[:], in0=env[:], in1=rd[:],
                                            op=mybir.AluOpType.mult)
                    # u[32j+n, 128k+e] = coef_n * d[(3k+j)*128+e] via K=3 matmuls
                    u = bigp.tile([D, 512], F32, tag="mm512")
                    for k in range(4):
                        nc.tensor.matmul(out=u[:, k * D:(k + 1) * D],
                                         lhsT=cm_sb[:, k * D:(k + 1) * D],
                                         rhs=dt3[:],
                                         start=True, stop=True)
                    kf = epool.tile([D, 512], F32, tag="kf")
                    nc.vector.tensor_scalar(out=kf[:], in0=u[:],
                                            scalar1=MAGIC, scalar2=MAGIC,
                                            op0=mybir.AluOpType.add,
                                            op1=mybir.AluOpType.subtract)
                    v = epool.tile([D, 512], F32, tag="v")
                    nc.vector.tensor_tensor(out=v[:], in0=u[:], in1=kf[:],
                                            op=mybir.AluOpType.subtract)
                    db = epool.tile([D, 512], F32, tag="db")
                    # full-tile copy seeds the raw-d rows (32j+20); Sin then
                    # overwrites rows 32j..32j+19 (ops must be 32-part aligned)
                    nc.vector.tensor_copy(out=db[:], in_=u[:])
                    for j in range(3):
                        nc.scalar.activation(
                            out=db[32 * j:32 * j + NB, :],
                            in_=v[32 * j:32 * j + NB, :],
                            func=mybir.ActivationFunctionType.Sin,
                            scale=float(2 * np.pi))
                    for t in range(12):
                        k, j = t // 3, t % 3
                        pw = smallp.tile([D, D], F32, tag="mm128")
                        nc.tensor.matmul(
                            out=pw[:],
                            lhsT=db[32 * j:32 * j + NB + 1, k * D:(k + 1) * D],
                            rhs=wext_sb[32 * j:32 * j + NB + 1, :],
                            start=True, stop=True)
                        ws = epool.tile([D, D], F32, tag="ws")
                        nc.scalar.activation(
                            out=ws[:], in_=pw[:],
                            func=mybir.ActivationFunctionType.Copy,
                            scale=scl[:, t:t + 1])
                        slot = c2 * 12 + t
                        nc.vector.tensor_tensor(
                            out=out_sb[:, slot, :], in0=ws[:],
                            in1=phi[:, slot, :], op=mybir.AluOpType.mult)
                nc.sync.dma_start(
                    out=out_dev[g * GCH:(g + 1) * GCH, :].rearrange(
                        "(s p) f -> p s f", p=D),
                    in_=out_sb[:])
    nc.finalize()
    return nc


_NC_CACHE = {}


def kernel(s_j, dist, nbrs, W1, b1, W2, b2, W_rbf, b_rbf):
    s_j = np.asarray(s_j, dtype=np.float32)
    dist = np.asarray(dist, dtype=np.float32)
    j_idx = np.asarray(nbrs)[:, 1].astype(np.int32)

    core = j_idx // NPC
    order = np.argsort(core, kind="stable")
    counts = np.bincount(core, minlength=N_CORES)
    n_g = int((counts.max() + GCH - 1) // GCH)
    E_pad = n_g * GCH

    w21 = np.concatenate([np.asarray(W_rbf, np.float32),
                          np.asarray(b_rbf, np.float32)[None, :]], axis=0)
    wext = np.zeros((D, D), dtype=np.float32)
    for qj in range(3):
        wext[32 * qj:32 * qj + NB + 1] = w21
    # cmat[t, 128k+32j+n] = delta(t, 3k+j) * coef_n
    # coef_n = (n+1)/10 (n<20), 1.0 (n=20, the raw-d row), 0 else
    coef = np.zeros(32, dtype=np.float32)
    coef[:NB] = (np.arange(NB) + 1) / 10.0
    coef[NB] = 1.0
    cmat = np.zeros((12, 512), dtype=np.float32)
    for k in range(4):
        for j in range(3):
            cmat[3 * k + j, 128 * k + 32 * j:128 * k + 32 * j + 32] = coef
    common = {
        "W1h": np.asarray(W1, np.float32).astype(np.float16),
        "W2h": np.asarray(W2, np.float32).astype(np.float16),
        "b1c": np.asarray(b1, np.float32).reshape(D, 1),
        "ones1": np.ones((1, D), dtype=np.float16),
        "b2r": np.asarray(b2, np.float32).astype(np.float16).reshape(1, D),
        "wext": wext,
        "cmat": cmat,
        "ident": np.eye(D, dtype=np.float32),
        "nhp": np.full((D, 1), -np.pi / 2, dtype=np.float32),
    }

    starts = np.zeros(N_CORES + 1, dtype=np.int64)
    starts[1:] = np.cumsum(counts)
    in_maps = []
    for c in range(N_CORES):
        sel = order[starts[c]:starts[c + 1]]
        n_c = counts[c]
        idx_pad = np.zeros(E_pad, dtype=np.int32)
        dist_pad = np.ones(E_pad, dtype=np.float32)
        idx_pad[:n_c] = j_idx[sel] - c * NPC
        dist_pad[:n_c] = dist[sel]
        s_T = np.zeros((D, NPT), dtype=np.float16)
        s_T[:, :NPC] = s_j[c * NPC:(c + 1) * NPC].T
        in_maps.append(dict(common, s_jT=s_T,
                            idx32=idx_pad.reshape(-1, 1),
                            dist3=dist_pad.reshape(-1, D)))

    if n_g not in _NC_CACHE:
        _NC_CACHE[n_g] = build_nc(n_g)
    nc = _NC_CACHE[n_g]

    res = run_bass_kernel_spmd(nc, in_maps, list(range(N_CORES)))
    valid = np.concatenate(
        [res.results[c]["out_dev"][:counts[c]] for c in range(N_CORES)], axis=0)
    out = np.empty((N_EDGES, D), dtype=np.float32)
    out[order] = valid.astype(np.float32)
    return out
